# revision 6
# baseline (speedup 1.0000x reference)
"""8-core Trainium2 Bass kernel for nn_DecoderLayer_50783693308094.

Decoder layer: pre-LN attention (with the source model's no-transpose reshape:
"head" n = token-block n of q/k/v reinterpreted (2048, 128); its output is
column block n of the attention output for all tokens) + pre-LN FFN.

Sharding (8 cores): token-parallel everywhere. Core c owns rows
[256c, 256(c+1)) of BOTH batches (= head-blocks 2c, 2c+1 of each batch). LN1 +
QKV + attention are fully local; one 8-core AllToAll redistributes On^T
column-slices; out-projection + residual + LN2 + FFN + residual are
token-local. Matmuls in bf16 (f32 accumulate), everything else f32.
"""
import os
import sys
import numpy as np

if "/opt/trn_rl_repo" not in sys.path:
    sys.path.insert(0, "/opt/trn_rl_repo")

import ml_dtypes
import concourse.bass as bass
import concourse.mybir as mybir
import concourse.tile as tile
from concourse.bass_utils import run_bass_kernel_spmd
from concourse.masks import make_identity
from contextlib import ExitStack

BF = ml_dtypes.bfloat16
F32 = mybir.dt.float32
BF16 = mybir.dt.bfloat16
B, S, D, M, H = 2, 2048, 2048, 8192, 16
d = 128
NC = 8
TOK = 512            # tokens per core
EPS = 1e-5
ISQ = float(1.0 / np.sqrt(np.float32(d)))

_CACHE = {}


def _ln_block(nc, pool, dst_bf16, src_f32, g_bc, b_bc, eps_t):
    """LayerNorm of one [128, D] f32 block -> bf16 into dst_bf16 (same shape)."""
    stats = pool.tile([128, 4, 6], F32, name="ln_stats", tag="ln_stats", bufs=2)
    src_g = src_f32.rearrange("p (s q) -> p s q", s=4)
    for sg in range(4):
        nc.vector.bn_stats(out=stats[:, sg, :], in_=src_g[:, sg, :])
    mv = pool.tile([128, 2], F32, name="ln_mv", tag="ln_mv", bufs=2)
    nc.vector.bn_aggr(out=mv, in_=stats)
    rstd = pool.tile([128, 1], F32, name="ln_rstd", tag="ln_rstd", bufs=2)
    nc.scalar.activation(out=rstd, in_=mv[:, 1:2],
                         func=mybir.ActivationFunctionType.Sqrt,
                         bias=eps_t, scale=1.0)
    nc.vector.reciprocal(out=rstd, in_=rstd)
    h0 = pool.tile([128, D], F32, name="ln_h0", tag="ln_h0", bufs=2)
    nc.vector.tensor_scalar(out=h0, in0=src_f32, scalar1=mv[:, 0:1], scalar2=rstd,
                            op0=mybir.AluOpType.subtract, op1=mybir.AluOpType.mult)
    nc.vector.tensor_mul(out=h0, in0=h0, in1=g_bc)
    nc.vector.tensor_add(out=dst_bf16, in0=h0, in1=b_bc)


def _build():
    nc = bass.Bass(num_devices=NC)

    x_in = nc.declare_dram_parameter("x", [TOK, D], F32, isOutput=False)
    wq_in = nc.declare_dram_parameter("wq", [16, 16, 128, 128], BF16, isOutput=False)
    wk_in = nc.declare_dram_parameter("wk", [16, 16, 128, 128], BF16, isOutput=False)
    wv_in = nc.declare_dram_parameter("wv", [D, D], BF16, isOutput=False)
    wo_in = nc.declare_dram_parameter("wo", [D, D], BF16, isOutput=False)
    w1_in = nc.declare_dram_parameter("w1", [64, 16, 128, 128], BF16, isOutput=False)
    w2_in = nc.declare_dram_parameter("w2", [M, D], BF16, isOutput=False)
    b1_in = nc.declare_dram_parameter("b1t", [128, 64], F32, isOutput=False)
    b2_in = nc.declare_dram_parameter("b2", [D], F32, isOutput=False)
    g1_in = nc.declare_dram_parameter("ln1g", [D], F32, isOutput=False)
    c1_in = nc.declare_dram_parameter("ln1b", [D], F32, isOutput=False)
    g2_in = nc.declare_dram_parameter("ln2g", [D], F32, isOutput=False)
    c2_in = nc.declare_dram_parameter("ln2b", [D], F32, isOutput=False)
    tri_in = nc.declare_dram_parameter("tri", [128, 128], F32, isOutput=False)
    out_ext = nc.declare_dram_parameter("out", [TOK, D], F32, isOutput=True)

    def bcast(ap_1d):
        return bass.AP(tensor=ap_1d.tensor, offset=ap_1d.offset,
                       ap=[[0, 128], list(ap_1d.ap[0])])

    with tile.TileContext(nc) as tc, ExitStack() as ctx:
        misc = ctx.enter_context(tc.tile_pool(name="misc", bufs=1))
        dram = ctx.enter_context(tc.tile_pool(name="dram", bufs=1, space="DRAM"))

        ident = misc.tile([128, 128], BF16)
        make_identity(nc, ident)
        tri_t = misc.tile([128, 128], F32)
        nc.sync.dma_start(out=tri_t, in_=tri_in[:, :])
        b1_t = misc.tile([128, 64], F32)
        nc.sync.dma_start(out=b1_t, in_=b1_in[:, :])
        eps_t = misc.tile([128, 1], F32)
        nc.vector.memset(eps_t, EPS)

        v_scr = dram.tile([TOK, D], BF16, name="v_scr")
        a2a_src = dram.tile([NC, 4, 128, 256], BF16, name="a2a_src")
        a2a_dst = dram.tile([NC, 4, 128, 256], BF16, name="a2a_dst")

        p_x2 = ctx.enter_context(tc.tile_pool(name="p_x2", bufs=1))
        x2_all = p_x2.tile([128, 4 * D], F32)

        p_cd = ctx.enter_context(tc.tile_pool(name="p_cd", bufs=1))
        h2T_all = p_cd.tile([128, 16 * TOK], BF16)

        with tc.tile_pool(name="p_x", bufs=1) as p_x:
            x_all = p_x.tile([128, 4 * D], F32)
            for tb in range(4):
                nc.sync.dma_start(out=x_all[:, tb * D:(tb + 1) * D],
                                  in_=x_in[tb * 128:(tb + 1) * 128, :])

            with tc.tile_pool(name="p_qk", bufs=1) as p_qk:
                qT_all = p_qk.tile([128, 16 * TOK], BF16)
                kT_all = p_qk.tile([128, 16 * TOK], BF16)

                # ---------------- phase A: LN1 + QKV ----------------
                with tc.tile_pool(name="p_a", bufs=1) as p_a, \
                     tc.tile_pool(name="ps_t", bufs=2, space="PSUM") as ps_t, \
                     tc.tile_pool(name="ps_qk", bufs=2, space="PSUM") as ps_qk, \
                     tc.tile_pool(name="ps_v", bufs=1, space="PSUM") as ps_v:
                    g1_t = p_a.tile([128, D], F32)
                    nc.sync.dma_start(out=g1_t, in_=bcast(g1_in[:]))
                    c1_t = p_a.tile([128, D], F32)
                    nc.sync.dma_start(out=c1_t, in_=bcast(c1_in[:]))

                    h_all = p_a.tile([128, 4 * D], BF16)
                    for tb in range(4):
                        _ln_block(nc, p_a, h_all[:, tb * D:(tb + 1) * D],
                                  x_all[:, tb * D:(tb + 1) * D], g1_t, c1_t, eps_t)

                    # hT_all[:, dc*512 + tb*128 + p_t] = h[t, dc*128+p]
                    hT_all = p_a.tile([128, 16 * TOK], BF16)
                    for tb in range(4):
                        for dg in range(4):
                            pt = ps_t.tile([128, 512], BF16, name="pt", tag="pt")
                            for z in range(4):
                                dc = dg * 4 + z
                                nc.tensor.transpose(
                                    pt[:, z * 128:(z + 1) * 128],
                                    h_all[:, tb * D + dc * 128: tb * D + (dc + 1) * 128],
                                    ident)
                            for z in range(4):
                                dc = dg * 4 + z
                                nc.scalar.copy(
                                    out=hT_all[:, dc * TOK + tb * 128: dc * TOK + (tb + 1) * 128],
                                    in_=pt[:, z * 128:(z + 1) * 128])

                    for w_dram, dstT in ((wq_in, qT_all), (wk_in, kT_all)):
                        for jc in range(16):
                            w_sb = p_a.tile([128, 16, 128], BF16, name="w_sb", tag="w_sb", bufs=3)
                            nc.sync.dma_start(out=w_sb,
                                              in_=w_dram[jc].rearrange("a p c -> p a c"))
                            pq = ps_qk.tile([128, TOK], F32, name="pq", tag="pq")
                            for dc in range(16):
                                nc.tensor.matmul(pq, w_sb[:, dc, :],
                                                 hT_all[:, dc * TOK:(dc + 1) * TOK],
                                                 start=(dc == 0), stop=(dc == 15))
                            nc.scalar.copy(out=dstT[:, jc * TOK:(jc + 1) * TOK], in_=pq)

                    for dv in range(4):
                        pv = [ps_v.tile([128, 512], F32, name=f"pv{tb}", tag=f"pv{tb}")
                              for tb in range(4)]
                        for dc in range(16):
                            wv_sb = p_a.tile([128, 512], BF16, name="wv_sb", tag="wv_sb", bufs=3)
                            nc.sync.dma_start(
                                out=wv_sb,
                                in_=wv_in[dc * 128:(dc + 1) * 128, dv * 512:(dv + 1) * 512])
                            for tb in range(4):
                                nc.tensor.matmul(pv[tb],
                                                 hT_all[:, dc * TOK + tb * 128: dc * TOK + (tb + 1) * 128],
                                                 wv_sb, start=(dc == 0), stop=(dc == 15))
                        for tb in range(4):
                            v_sb = p_a.tile([128, 512], BF16, name="v_sb", tag="v_sb", bufs=2)
                            nc.scalar.copy(out=v_sb, in_=pv[tb])
                            nc.sync.dma_start(
                                out=v_scr[tb * 128:(tb + 1) * 128, dv * 512:(dv + 1) * 512],
                                in_=v_sb)

                # ---------------- phase B: attention ----------------
                qT_r = qT_all.rearrange("p (jc t) -> p t jc", jc=16)
                kT_r = kT_all.rearrange("p (jc t) -> p t jc", jc=16)
                v_flat = v_scr.rearrange("a b -> (a b)")
                with tc.tile_pool(name="p_b", bufs=1) as p_b, \
                     tc.tile_pool(name="ps_s", bufs=2, space="PSUM") as ps_s, \
                     tc.tile_pool(name="ps_pt", bufs=2, space="PSUM") as ps_pt, \
                     tc.tile_pool(name="ps_o", bufs=2, space="PSUM") as ps_o:
                    for n in range(4):
                        # materialize interleaved QnT/KnT [dd, i = r*16+jc]
                        qnT = p_b.tile([128, 2048], BF16, name="qnT", tag="qnT", bufs=2)
                        nc.vector.tensor_copy(
                            out=qnT.rearrange("p (r jc) -> p r jc", jc=16),
                            in_=qT_r[:, n * 128:(n + 1) * 128, :])
                        knT = p_b.tile([128, 2048], BF16, name="knT", tag="knT", bufs=2)
                        nc.vector.tensor_copy(
                            out=knT.rearrange("p (r jc) -> p r jc", jc=16),
                            in_=kT_r[:, n * 128:(n + 1) * 128, :])
                        vn_sb = p_b.tile([128, 16 * 128], BF16, name="vn_sb", tag="vn", bufs=2)
                        for kb in range(16):
                            blk = v_flat[n * 128 * D + kb * 16384:
                                         n * 128 * D + (kb + 1) * 16384]
                            nc.sync.dma_start(
                                out=vn_sb[:, kb * 128:(kb + 1) * 128],
                                in_=blk.rearrange("(p c) -> p c", p=128))
                        onT = p_b.tile([128, 2048], BF16, name="onT", tag="onT", bufs=2)
                        for qi in range(16):
                            L = (qi + 1) * 128
                            lhs_q = qnT[:, qi * 128:(qi + 1) * 128]
                            s_all = p_b.tile([128, 2048], F32, name="s_all", tag="s_all", bufs=2)
                            for kc in range(0, L, 512):
                                ncols = min(512, L - kc)
                                ps = ps_s.tile([128, 512], F32, name="ps", tag="ps")
                                nc.tensor.matmul(ps[:, 0:ncols], lhs_q,
                                                 knT[:, kc:kc + ncols],
                                                 start=True, stop=True)
                                nc.vector.tensor_copy(s_all[:, kc:kc + ncols],
                                                      ps[:, 0:ncols])
                            nc.vector.tensor_add(out=s_all[:, qi * 128:L],
                                                 in0=s_all[:, qi * 128:L], in1=tri_t)
                            mx = p_b.tile([128, 1], F32, name="mx", tag="mx", bufs=2)
                            nc.vector.reduce_max(mx, s_all[:, 0:L],
                                                 axis=mybir.AxisListType.X)
                            nm = p_b.tile([128, 1], F32, name="nm", tag="nm", bufs=2)
                            nc.vector.tensor_scalar_mul(nm, mx, -ISQ)
                            p_all = p_b.tile([128, 2048], BF16, name="p_all", tag="p_all", bufs=2)
                            rs = p_b.tile([128, 1], F32, name="rs", tag="rs", bufs=2)
                            nc.scalar.activation(out=p_all[:, 0:L], in_=s_all[:, 0:L],
                                                 func=mybir.ActivationFunctionType.Exp,
                                                 bias=nm, scale=ISQ, accum_out=rs)
                            rinv = p_b.tile([128, 1], F32, name="rinv", tag="rinv", bufs=2)
                            nc.vector.reciprocal(out=rinv, in_=rs)
                            nc.vector.tensor_scalar_mul(p_all[:, 0:L], p_all[:, 0:L], rinv)
                            pT_all = p_b.tile([128, 2048], BF16, name="pT_all", tag="pT_all", bufs=2)
                            for grp in range(0, qi + 1, 4):
                                gn = min(4, qi + 1 - grp)
                                pt2 = ps_pt.tile([128, 512], BF16, name="pt2", tag="pt2")
                                for z in range(gn):
                                    nc.tensor.transpose(
                                        pt2[:, z * 128:(z + 1) * 128],
                                        p_all[:, (grp + z) * 128:(grp + z + 1) * 128],
                                        ident)
                                nc.scalar.copy(
                                    out=pT_all[:, grp * 128:(grp + gn) * 128],
                                    in_=pt2[:, 0:gn * 128])
                            po = ps_o.tile([128, 128], F32, name="po", tag="po")
                            for kb in range(qi + 1):
                                nc.tensor.matmul(po, vn_sb[:, kb * 128:(kb + 1) * 128],
                                                 pT_all[:, kb * 128:(kb + 1) * 128],
                                                 start=(kb == 0), stop=(kb == qi))
                            nc.scalar.copy(out=onT[:, qi * 128:(qi + 1) * 128], in_=po)
                        for j in range(NC):
                            nc.sync.dma_start(out=a2a_src[j, n],
                                              in_=onT[:, j * 256:(j + 1) * 256])

            # ---------------- AllToAll (grouped per batch) ----------------
            nc.gpsimd.collective_compute(
                "AllToAll", mybir.AluOpType.bypass,
                replica_groups=[list(range(NC))],
                ins=[a2a_src[:, :, :, :]], outs=[a2a_dst[:, :, :, :]])

            # ---------------- phase C: out-proj + residual + LN2 ----------
            with tc.tile_pool(name="p_c", bufs=1) as p_c:
                oT_sb = p_c.tile([128, 16 * TOK], BF16)
                for hc in range(16):
                    for b_ in range(2):
                        nc.sync.dma_start(
                            out=oT_sb[:, hc * TOK + b_ * 256: hc * TOK + (b_ + 1) * 256],
                            in_=a2a_dst[hc // 2, b_ * 2 + hc % 2])
                ps_c_ctx = tc.tile_pool(name="ps_c", bufs=1, space="PSUM")
                ps_c = ps_c_ctx.__enter__()
                for dh in range(2):
                    wo_sb = p_c.tile([128, 16 * 1024], BF16, name="wo_sb", tag="wo_sb", bufs=1)
                    for hc in range(16):
                        nc.sync.dma_start(
                            out=wo_sb[:, hc * 1024:(hc + 1) * 1024],
                            in_=wo_in[hc * 128:(hc + 1) * 128, dh * 1024:(dh + 1) * 1024])
                    pc = [ps_c.tile([128, 512], F32, name=f"pc{i}", tag=f"pc{i}")
                          for i in range(8)]
                    for hc in range(16):
                        for tb in range(4):
                            for ds in range(2):
                                nc.tensor.matmul(
                                    pc[tb * 2 + ds],
                                    oT_sb[:, hc * TOK + tb * 128: hc * TOK + (tb + 1) * 128],
                                    wo_sb[:, hc * 1024 + ds * 512: hc * 1024 + (ds + 1) * 512],
                                    start=(hc == 0), stop=(hc == 15))
                    for tb in range(4):
                        for ds in range(2):
                            dout = dh * 2 + ds
                            sl_ = slice(tb * D + dout * 512, tb * D + (dout + 1) * 512)
                            nc.vector.tensor_add(out=x2_all[:, sl_],
                                                 in0=pc[tb * 2 + ds], in1=x_all[:, sl_])

                ps_c_ctx.__exit__(None, None, None)

                g2_t = p_c.tile([128, D], F32)
                nc.sync.dma_start(out=g2_t, in_=bcast(g2_in[:]))
                c2_t = p_c.tile([128, D], F32)
                nc.sync.dma_start(out=c2_t, in_=bcast(c2_in[:]))
                b2_t = p_c.tile([128, D], F32)
                nc.sync.dma_start(out=b2_t, in_=bcast(b2_in[:]))

                h2_all = p_c.tile([128, 4 * D], BF16)
                for tb in range(4):
                    _ln_block(nc, p_c, h2_all[:, tb * D:(tb + 1) * D],
                              x2_all[:, tb * D:(tb + 1) * D], g2_t, c2_t, eps_t)
                # fold b2 into x2 now (x2b = x2 + b2)
                for tb in range(4):
                    nc.vector.tensor_add(out=x2_all[:, tb * D:(tb + 1) * D],
                                         in0=x2_all[:, tb * D:(tb + 1) * D], in1=b2_t)
                with tc.tile_pool(name="ps_t2", bufs=2, space="PSUM") as ps_t2:
                    for tb in range(4):
                        for dg in range(4):
                            pt3 = ps_t2.tile([128, 512], BF16, name="pt3", tag="pt3")
                            for z in range(4):
                                dc = dg * 4 + z
                                nc.tensor.transpose(
                                    pt3[:, z * 128:(z + 1) * 128],
                                    h2_all[:, tb * D + dc * 128: tb * D + (dc + 1) * 128],
                                    ident)
                            for z in range(4):
                                dc = dg * 4 + z
                                nc.scalar.copy(
                                    out=h2T_all[:, dc * TOK + tb * 128: dc * TOK + (tb + 1) * 128],
                                    in_=pt3[:, z * 128:(z + 1) * 128])

        # ---------------- phase D: FFN ----------------
        with tc.tile_pool(name="p_d", bufs=1) as p_d:
            ff1T = p_d.tile([128, 64 * TOK], BF16)
            with tc.tile_pool(name="ps_f1", bufs=2, space="PSUM") as ps_f1:
                for m in range(64):
                    w1_sb = p_d.tile([128, 16, 128], BF16, name="w1_sb", tag="w1_sb", bufs=3)
                    nc.sync.dma_start(out=w1_sb, in_=w1_in[m].rearrange("a p c -> p a c"))
                    pf = ps_f1.tile([128, TOK], F32, name="pf", tag="pf")
                    for dc in range(16):
                        nc.tensor.matmul(pf, w1_sb[:, dc, :],
                                         h2T_all[:, dc * TOK:(dc + 1) * TOK],
                                         start=(dc == 0), stop=(dc == 15))
                    nc.scalar.activation(out=ff1T[:, m * TOK:(m + 1) * TOK], in_=pf,
                                         func=mybir.ActivationFunctionType.Relu,
                                         bias=b1_t[:, m:m + 1], scale=1.0)
            with tc.tile_pool(name="ps_f2", bufs=1, space="PSUM") as ps_f2:
                for dh in range(2):
                    pd = [ps_f2.tile([128, 512], F32, name=f"pd{i}", tag=f"pd{i}")
                          for i in range(8)]
                    for m in range(64):
                        w2_sb = p_d.tile([128, 1024], BF16, name="w2_sb", tag="w2_sb", bufs=3)
                        nc.sync.dma_start(
                            out=w2_sb,
                            in_=w2_in[m * 128:(m + 1) * 128, dh * 1024:(dh + 1) * 1024])
                        for tb in range(4):
                            for ds in range(2):
                                nc.tensor.matmul(
                                    pd[tb * 2 + ds],
                                    ff1T[:, m * TOK + tb * 128: m * TOK + (tb + 1) * 128],
                                    w2_sb[:, ds * 512:(ds + 1) * 512],
                                    start=(m == 0), stop=(m == 63))
                    for tb in range(4):
                        for ds in range(2):
                            dout = dh * 2 + ds
                            o_sb = p_d.tile([128, 512], F32, name="o_sb", tag="o_sb", bufs=2)
                            nc.vector.tensor_add(
                                out=o_sb, in0=pd[tb * 2 + ds],
                                in1=x2_all[:, tb * D + dout * 512: tb * D + (dout + 1) * 512])
                            nc.sync.dma_start(
                                out=out_ext[tb * 128:(tb + 1) * 128,
                                            dout * 512:(dout + 1) * 512],
                                in_=o_sb)

    # legalize: this walrus build accepts at most ONE sync-wait per instruction
    _legalize_waits(nc, max_waits=1)
    return nc


def _legalize_waits(nc, max_waits=1):
    f = nc.m.functions[0]
    n_split = 0
    for blk in f.blocks:
        insts = blk.instructions
        out = []
        changed = False
        for inst in insts:
            si = inst.sync_info
            if si is not None and si.on_wait and len(si.on_wait) > max_waits:
                waits = list(si.on_wait)
                for w in waits[:-max_waits]:
                    nop = mybir.InstNoOp(
                        name=nc.get_next_instruction_name(),
                        engine=inst.engine,
                        sync_info=mybir.SyncInfo(on_wait=[w], on_update=[]),
                        bass_nofuse=True,
                    )
                    nc.register_instruction(nop)
                    out.append(nop)
                    n_split += 1
                inst.sync_info = mybir.SyncInfo(
                    on_wait=waits[-max_waits:], on_update=list(si.on_update or []))
                changed = True
            out.append(inst)
        if changed:
            blk.instructions = out
    return n_split


def _get_nc():
    if "nc" not in _CACHE:
        _CACHE["nc"] = _build()
    return _CACHE["nc"]


def kernel(**inputs):
    x = np.ascontiguousarray(np.asarray(inputs["x"], np.float32).reshape(B * S, D))
    Wq = np.asarray(inputs["Wq"], np.float32)
    Wk = np.asarray(inputs["Wk"], np.float32)
    Wv = np.asarray(inputs["Wv"], np.float32)
    Wo = np.asarray(inputs["Wo"], np.float32)
    W1 = np.asarray(inputs["W1"], np.float32)
    W2 = np.asarray(inputs["W2"], np.float32)
    b1 = np.asarray(inputs["b1"], np.float32)
    b2 = np.asarray(inputs["b2"], np.float32)
    g1 = np.asarray(inputs["ln1_g"], np.float32)
    c1 = np.asarray(inputs["ln1_b"], np.float32)
    g2 = np.asarray(inputs["ln2_g"], np.float32)
    c2 = np.asarray(inputs["ln2_b"], np.float32)

    wq_p = np.ascontiguousarray(Wq.reshape(16, 128, 16, 128).transpose(2, 0, 1, 3)).astype(BF)
    wk_p = np.ascontiguousarray(Wk.reshape(16, 128, 16, 128).transpose(2, 0, 1, 3)).astype(BF)
    w1_p = np.ascontiguousarray(W1.reshape(16, 128, 64, 128).transpose(2, 0, 1, 3)).astype(BF)
    wv_b = Wv.astype(BF)
    wo_b = Wo.astype(BF)
    w2_b = W2.astype(BF)
    b1t = np.ascontiguousarray(b1.reshape(64, 128).T)
    tri = np.triu(np.full((128, 128), np.float32(-1e9), np.float32), k=1)

    shared = dict(wq=wq_p, wk=wk_p, wv=wv_b, wo=wo_b, w1=w1_p, w2=w2_b,
                  b1t=b1t, b2=b2, ln1g=g1, ln1b=c1, ln2g=g2, ln2b=c2, tri=tri)
    xb = x.reshape(B, S, D)
    in_maps = [dict(shared, x=np.ascontiguousarray(
                   np.concatenate([xb[0, 256 * c:256 * (c + 1)],
                                   xb[1, 256 * c:256 * (c + 1)]], axis=0)))
               for c in range(NC)]

    nc = _get_nc()
    trace = bool(int(os.environ.get("BASS_KERNEL_TRACE", "0")))
    res = run_bass_kernel_spmd(nc, in_maps, list(range(NC)), trace=trace)
    if trace:
        _CACHE["last_exec_time_ns"] = res.exec_time_ns
        _CACHE["last_results"] = res
    out = np.empty((B, S, D), np.float32)
    for c in range(NC):
        oc = res.results[c]["out"]
        out[0, 256 * c:256 * (c + 1)] = oc[0:256]
        out[1, 256 * c:256 * (c + 1)] = oc[256:512]
    return out


# revision 8
# speedup vs baseline: 1.0940x; 1.0940x over previous
"""8-core Trainium2 Bass kernel for nn_DecoderLayer_50783693308094.

Decoder layer: pre-LN attention (with the source model's no-transpose reshape:
"head" n = token-block n of q/k/v reinterpreted (2048, 128); its output is
column block n of the attention output for all tokens) + pre-LN FFN.

Sharding (8 cores): token-parallel everywhere. Core c owns rows
[256c, 256(c+1)) of BOTH batches (= head-blocks 2c, 2c+1 of each batch). LN1 +
QKV + attention are fully local; one 8-core AllToAll redistributes On^T
column-slices; out-projection + residual + LN2 + FFN + residual are
token-local. Matmuls in bf16 (f32 accumulate), everything else f32.
"""
import os
import sys
import numpy as np

if "/opt/trn_rl_repo" not in sys.path:
    sys.path.insert(0, "/opt/trn_rl_repo")

import ml_dtypes
import concourse.bass as bass
import concourse.mybir as mybir
import concourse.tile as tile
from concourse.bass_utils import run_bass_kernel_spmd
from concourse.masks import make_identity
from contextlib import ExitStack

BF = ml_dtypes.bfloat16
F32 = mybir.dt.float32
BF16 = mybir.dt.bfloat16
B, S, D, M, H = 2, 2048, 2048, 8192, 16
d = 128
NC = 8
TOK = 512            # tokens per core
EPS = 1e-5
ISQ = float(1.0 / np.sqrt(np.float32(d)))

_CACHE = {}


def _ln_block(nc, pool, dst_bf16, src_f32, g_bc, b_bc, eps_t):
    """LayerNorm of one [128, D] f32 block -> bf16 into dst_bf16 (same shape)."""
    stats = pool.tile([128, 4, 6], F32, name="ln_stats", tag="ln_stats", bufs=2)
    src_g = src_f32.rearrange("p (s q) -> p s q", s=4)
    for sg in range(4):
        nc.vector.bn_stats(out=stats[:, sg, :], in_=src_g[:, sg, :])
    mv = pool.tile([128, 2], F32, name="ln_mv", tag="ln_mv", bufs=2)
    nc.vector.bn_aggr(out=mv, in_=stats)
    rstd = pool.tile([128, 1], F32, name="ln_rstd", tag="ln_rstd", bufs=2)
    nc.scalar.activation(out=rstd, in_=mv[:, 1:2],
                         func=mybir.ActivationFunctionType.Sqrt,
                         bias=eps_t, scale=1.0)
    nc.vector.reciprocal(out=rstd, in_=rstd)
    h0 = pool.tile([128, D], F32, name="ln_h0", tag="ln_h0", bufs=2)
    nc.vector.tensor_scalar(out=h0, in0=src_f32, scalar1=mv[:, 0:1], scalar2=rstd,
                            op0=mybir.AluOpType.subtract, op1=mybir.AluOpType.mult)
    nc.vector.tensor_mul(out=h0, in0=h0, in1=g_bc)
    nc.vector.tensor_add(out=dst_bf16, in0=h0, in1=b_bc)


def _build():
    nc = bass.Bass(num_devices=NC)

    x_in = nc.declare_dram_parameter("x", [TOK, D], F32, isOutput=False)
    wq_in = nc.declare_dram_parameter("wq", [16, 16, 128, 128], BF16, isOutput=False)
    wk_in = nc.declare_dram_parameter("wk", [16, 16, 128, 128], BF16, isOutput=False)
    wv_in = nc.declare_dram_parameter("wv", [D, D], BF16, isOutput=False)
    wo_in = nc.declare_dram_parameter("wo", [D, D], BF16, isOutput=False)
    w1_in = nc.declare_dram_parameter("w1", [64, 16, 128, 128], BF16, isOutput=False)
    w2_in = nc.declare_dram_parameter("w2", [M, D], BF16, isOutput=False)
    b1_in = nc.declare_dram_parameter("b1t", [128, 64], F32, isOutput=False)
    b2_in = nc.declare_dram_parameter("b2", [D], F32, isOutput=False)
    g1_in = nc.declare_dram_parameter("ln1g", [D], F32, isOutput=False)
    c1_in = nc.declare_dram_parameter("ln1b", [D], F32, isOutput=False)
    g2_in = nc.declare_dram_parameter("ln2g", [D], F32, isOutput=False)
    c2_in = nc.declare_dram_parameter("ln2b", [D], F32, isOutput=False)
    tri_in = nc.declare_dram_parameter("tri", [128, 128], F32, isOutput=False)
    out_ext = nc.declare_dram_parameter("out", [TOK, D], F32, isOutput=True)

    def bcast(ap_1d):
        return bass.AP(tensor=ap_1d.tensor, offset=ap_1d.offset,
                       ap=[[0, 128], list(ap_1d.ap[0])])

    with tile.TileContext(nc) as tc, ExitStack() as ctx:
        misc = ctx.enter_context(tc.tile_pool(name="misc", bufs=1))
        dram = ctx.enter_context(tc.tile_pool(name="dram", bufs=1, space="DRAM"))

        ident = misc.tile([128, 128], BF16)
        make_identity(nc, ident)
        tri_t = misc.tile([128, 128], F32)
        nc.sync.dma_start(out=tri_t, in_=tri_in[:, :])
        b1_t = misc.tile([128, 64], F32)
        nc.sync.dma_start(out=b1_t, in_=b1_in[:, :])
        eps_t = misc.tile([128, 1], F32)
        nc.vector.memset(eps_t, EPS)

        v_scr = dram.tile([TOK, D], BF16, name="v_scr")
        a2a_src0 = dram.tile([NC, 2, 128, 256], BF16, name="a2a_src0")
        a2a_dst0 = dram.tile([NC, 2, 128, 256], BF16, name="a2a_dst0")
        a2a_src1 = dram.tile([NC, 2, 128, 256], BF16, name="a2a_src1")
        a2a_dst1 = dram.tile([NC, 2, 128, 256], BF16, name="a2a_dst1")

        p_x2 = ctx.enter_context(tc.tile_pool(name="p_x2", bufs=1))
        x2_all = p_x2.tile([128, 4 * D], F32)

        p_cd = ctx.enter_context(tc.tile_pool(name="p_cd", bufs=1))
        h2T_all = p_cd.tile([128, 16 * TOK], BF16)

        if True:
            with tc.tile_pool(name="p_qk", bufs=1) as p_qk:
                qT_all = p_qk.tile([128, 16 * TOK], BF16)
                kT_all = p_qk.tile([128, 16 * TOK], BF16)

                # ---------------- phase A: LN1 + QKV ----------------
                with tc.tile_pool(name="p_a", bufs=1) as p_a, \
                     tc.tile_pool(name="ps_t", bufs=2, space="PSUM") as ps_t, \
                     tc.tile_pool(name="ps_qk", bufs=2, space="PSUM") as ps_qk, \
                     tc.tile_pool(name="ps_v", bufs=1, space="PSUM") as ps_v:
                    x_t = []
                    for tb in range(4):
                        xt = p_a.tile([128, D], F32, name=f"x_t{tb}")
                        nc.sync.dma_start(out=xt, in_=x_in[tb * 128:(tb + 1) * 128, :])
                        x_t.append(xt)
                    g1_t = p_a.tile([128, D], F32)
                    nc.sync.dma_start(out=g1_t, in_=bcast(g1_in[:]))
                    c1_t = p_a.tile([128, D], F32)
                    nc.sync.dma_start(out=c1_t, in_=bcast(c1_in[:]))

                    h_t = []
                    for tb in range(4):
                        ht = p_a.tile([128, D], BF16, name=f"h_t{tb}")
                        _ln_block(nc, p_a, ht, x_t[tb], g1_t, c1_t, eps_t)
                        h_t.append(ht)

                    # hT_all[:, dc*512 + tb*128 + p_t] = h[t, dc*128+p]
                    hT_all = p_a.tile([128, 16 * TOK], BF16)
                    for tb in range(4):
                        for dg in range(4):
                            pt = ps_t.tile([128, 512], BF16, name="pt", tag="pt")
                            for z in range(4):
                                dc = dg * 4 + z
                                nc.tensor.transpose(
                                    pt[:, z * 128:(z + 1) * 128],
                                    h_t[tb][:, dc * 128:(dc + 1) * 128],
                                    ident)
                            for z in range(4):
                                dc = dg * 4 + z
                                nc.scalar.copy(
                                    out=hT_all[:, dc * TOK + tb * 128: dc * TOK + (tb + 1) * 128],
                                    in_=pt[:, z * 128:(z + 1) * 128])

                    for w_dram, dstT in ((wq_in, qT_all), (wk_in, kT_all)):
                        for jc in range(16):
                            w_sb = p_a.tile([128, 16, 128], BF16, name="w_sb", tag="w_sb", bufs=3)
                            nc.sync.dma_start(out=w_sb,
                                              in_=w_dram[jc].rearrange("a p c -> p a c"))
                            pq = ps_qk.tile([128, TOK], F32, name="pq", tag="pq")
                            for dc in range(16):
                                nc.tensor.matmul(pq, w_sb[:, dc, :],
                                                 hT_all[:, dc * TOK:(dc + 1) * TOK],
                                                 start=(dc == 0), stop=(dc == 15))
                            nc.scalar.copy(out=dstT[:, jc * TOK:(jc + 1) * TOK], in_=pq)

                    for dv in range(4):
                        pv = [ps_v.tile([128, 512], F32, name=f"pv{tb}", tag=f"pv{tb}")
                              for tb in range(4)]
                        for dc in range(16):
                            wv_sb = p_a.tile([128, 512], BF16, name="wv_sb", tag="wv_sb", bufs=3)
                            nc.sync.dma_start(
                                out=wv_sb,
                                in_=wv_in[dc * 128:(dc + 1) * 128, dv * 512:(dv + 1) * 512])
                            for tb in range(4):
                                nc.tensor.matmul(pv[tb],
                                                 hT_all[:, dc * TOK + tb * 128: dc * TOK + (tb + 1) * 128],
                                                 wv_sb, start=(dc == 0), stop=(dc == 15))
                        for tb in range(4):
                            v_sb = p_a.tile([128, 512], BF16, name="v_sb", tag="v_sb", bufs=2)
                            nc.scalar.copy(out=v_sb, in_=pv[tb])
                            nc.sync.dma_start(
                                out=v_scr[tb * 128:(tb + 1) * 128, dv * 512:(dv + 1) * 512],
                                in_=v_sb)

                # ---------------- phase B: attention ----------------
                qT_r = qT_all.rearrange("p (jc t) -> p t jc", jc=16)
                kT_r = kT_all.rearrange("p (jc t) -> p t jc", jc=16)
                v_flat = v_scr.rearrange("a b -> (a b)")
                with tc.tile_pool(name="ps_proj", bufs=1, space="PSUM") as ps_proj, \
                     tc.tile_pool(name="p_b", bufs=1) as p_b, \
                     tc.tile_pool(name="ps_s", bufs=2, space="PSUM") as ps_s, \
                     tc.tile_pool(name="ps_pt", bufs=2, space="PSUM") as ps_pt, \
                     tc.tile_pool(name="ps_o", bufs=2, space="PSUM") as ps_o:
                    for n in range(4):
                        # materialize interleaved QnT/KnT [dd, i = r*16+jc]
                        qnT = p_b.tile([128, 2048], BF16, name="qnT", tag="qnT", bufs=2)
                        nc.vector.tensor_copy(
                            out=qnT.rearrange("p (r jc) -> p r jc", jc=16),
                            in_=qT_r[:, n * 128:(n + 1) * 128, :])
                        knT = p_b.tile([128, 2048], BF16, name="knT", tag="knT", bufs=2)
                        nc.vector.tensor_copy(
                            out=knT.rearrange("p (r jc) -> p r jc", jc=16),
                            in_=kT_r[:, n * 128:(n + 1) * 128, :])
                        vn_sb = p_b.tile([128, 16 * 128], BF16, name="vn_sb", tag="vn", bufs=2)
                        for kb in range(16):
                            blk = v_flat[n * 128 * D + kb * 16384:
                                         n * 128 * D + (kb + 1) * 16384]
                            nc.sync.dma_start(
                                out=vn_sb[:, kb * 128:(kb + 1) * 128],
                                in_=blk.rearrange("(p c) -> p c", p=128))
                        onT = p_b.tile([128, 2048], BF16, name="onT", tag="onT", bufs=2)
                        for qi in range(16):
                            L = (qi + 1) * 128
                            nch = qi // 4 + 1
                            lhs_q = qnT[:, qi * 128:(qi + 1) * 128]
                            p_all = p_b.tile([128, 2048], BF16, name="p_all", tag="p_all", bufs=2)
                            rsp = p_b.tile([128, 4], F32, name="rsp", tag="rsp", bufs=2)
                            for ci, kc in enumerate(range(0, L, 512)):
                                ncols = min(512, L - kc)
                                ps = ps_s.tile([128, 512], F32, name="ps", tag="ps")
                                nc.tensor.matmul(ps[:, 0:ncols], lhs_q,
                                                 knT[:, kc:kc + ncols],
                                                 start=True, stop=True)
                                if ci == nch - 1:
                                    off = qi * 128 - kc
                                    nc.vector.tensor_add(out=ps[:, off:off + 128],
                                                         in0=ps[:, off:off + 128],
                                                         in1=tri_t)
                                nc.scalar.activation(
                                    out=p_all[:, kc:kc + ncols], in_=ps[:, 0:ncols],
                                    func=mybir.ActivationFunctionType.Exp,
                                    bias=0.0, scale=ISQ,
                                    accum_out=rsp[:, ci:ci + 1])
                            rinv = p_b.tile([128, 1], F32, name="rinv", tag="rinv", bufs=2)
                            nc.vector.reduce_sum(rinv, rsp[:, 0:nch],
                                                 axis=mybir.AxisListType.X)
                            nc.vector.reciprocal(out=rinv, in_=rinv)
                            nc.vector.tensor_scalar_mul(p_all[:, 0:L], p_all[:, 0:L], rinv)
                            pT_all = p_b.tile([128, 2048], BF16, name="pT_all", tag="pT_all", bufs=2)
                            for grp in range(0, qi + 1, 4):
                                gn = min(4, qi + 1 - grp)
                                pt2 = ps_pt.tile([128, 512], BF16, name="pt2", tag="pt2")
                                for z in range(gn):
                                    nc.tensor.transpose(
                                        pt2[:, z * 128:(z + 1) * 128],
                                        p_all[:, (grp + z) * 128:(grp + z + 1) * 128],
                                        ident)
                                nc.vector.tensor_copy(
                                    out=pT_all[:, grp * 128:(grp + gn) * 128],
                                    in_=pt2[:, 0:gn * 128])
                            po = ps_o.tile([128, 128], F32, name="po", tag="po")
                            for kb in range(qi + 1):
                                nc.tensor.matmul(po, vn_sb[:, kb * 128:(kb + 1) * 128],
                                                 pT_all[:, kb * 128:(kb + 1) * 128],
                                                 start=(kb == 0), stop=(kb == qi))
                            nc.scalar.copy(out=onT[:, qi * 128:(qi + 1) * 128], in_=po)
                        a2a_s = a2a_src0 if n < 2 else a2a_src1
                        for j in range(NC):
                            nc.sync.dma_start(out=a2a_s[j, n % 2],
                                              in_=onT[:, j * 256:(j + 1) * 256])
                        if n == 1:
                            nc.gpsimd.collective_compute(
                                "AllToAll", mybir.AluOpType.bypass,
                                replica_groups=[list(range(NC))],
                                ins=[a2a_src0[:, :, :, :]], outs=[a2a_dst0[:, :, :, :]])
                        if n == 3:
                            nc.gpsimd.collective_compute(
                                "AllToAll", mybir.AluOpType.bypass,
                                replica_groups=[list(range(NC))],
                                ins=[a2a_src1[:, :, :, :]], outs=[a2a_dst1[:, :, :, :]])

                    # ------------ phase C proj (inside ps_proj scope) ------------
                    with tc.tile_pool(name="p_c1", bufs=1) as p_c1:
                        wo_done = []
                        for bh in range(2):
                            a2a_d = a2a_dst0 if bh == 0 else a2a_dst1
                            oT_sbh = p_c1.tile([128, 16 * 256], BF16, name=f"oT_sb{bh}")
                            for hc in range(16):
                                nc.sync.dma_start(
                                    out=oT_sbh[:, hc * 256:(hc + 1) * 256],
                                    in_=a2a_d[hc // 2, hc % 2])
                            for dout in range(4):
                                wo_sb = p_c1.tile([128, 16 * 512], BF16,
                                                  name="wo_sb", tag="wo_sb", bufs=2)
                                for hc in range(16):
                                    nc.sync.dma_start(
                                        out=wo_sb[:, hc * 512:(hc + 1) * 512],
                                        in_=wo_in[hc * 128:(hc + 1) * 128,
                                                  dout * 512:(dout + 1) * 512])
                                for tbh in range(2):
                                    tb = bh * 2 + tbh
                                    pc = ps_proj.tile([128, 512], F32,
                                                      name="pc", tag="pc", bufs=2)
                                    for hc in range(16):
                                        nc.tensor.matmul(
                                            pc,
                                            oT_sbh[:, hc * 256 + tbh * 128: hc * 256 + tbh * 128 + 128],
                                            wo_sb[:, hc * 512:(hc + 1) * 512],
                                            start=(hc == 0), stop=(hc == 15))
                                    xr = p_c1.tile([128, 512], F32, name="xr", tag="xr", bufs=2)
                                    nc.sync.dma_start(
                                        out=xr,
                                        in_=x_in[tb * 128:(tb + 1) * 128,
                                                 dout * 512:(dout + 1) * 512])
                                    sl_ = slice(tb * D + dout * 512,
                                                tb * D + (dout + 1) * 512)
                                    nc.vector.tensor_add(out=x2_all[:, sl_],
                                                         in0=pc, in1=xr)

            # ---------------- phase C: LN2 + h2T ----------------
            with tc.tile_pool(name="p_c", bufs=1) as p_c:
                g2_t = p_c.tile([128, D], F32)
                nc.sync.dma_start(out=g2_t, in_=bcast(g2_in[:]))
                c2_t = p_c.tile([128, D], F32)
                nc.sync.dma_start(out=c2_t, in_=bcast(c2_in[:]))
                b2_t = p_c.tile([128, D], F32)
                nc.sync.dma_start(out=b2_t, in_=bcast(b2_in[:]))

                h2_all = p_c.tile([128, 4 * D], BF16)
                for tb in range(4):
                    _ln_block(nc, p_c, h2_all[:, tb * D:(tb + 1) * D],
                              x2_all[:, tb * D:(tb + 1) * D], g2_t, c2_t, eps_t)
                # fold b2 into x2 now (x2b = x2 + b2)
                for tb in range(4):
                    nc.vector.tensor_add(out=x2_all[:, tb * D:(tb + 1) * D],
                                         in0=x2_all[:, tb * D:(tb + 1) * D], in1=b2_t)
                with tc.tile_pool(name="ps_t2", bufs=2, space="PSUM") as ps_t2:
                    for tb in range(4):
                        for dg in range(4):
                            pt3 = ps_t2.tile([128, 512], BF16, name="pt3", tag="pt3")
                            for z in range(4):
                                dc = dg * 4 + z
                                nc.tensor.transpose(
                                    pt3[:, z * 128:(z + 1) * 128],
                                    h2_all[:, tb * D + dc * 128: tb * D + (dc + 1) * 128],
                                    ident)
                            for z in range(4):
                                dc = dg * 4 + z
                                nc.scalar.copy(
                                    out=h2T_all[:, dc * TOK + tb * 128: dc * TOK + (tb + 1) * 128],
                                    in_=pt3[:, z * 128:(z + 1) * 128])

        # ---------------- phase D: FFN ----------------
        with tc.tile_pool(name="p_d", bufs=1) as p_d:
            ff1T = p_d.tile([128, 64 * TOK], BF16)
            with tc.tile_pool(name="ps_f1", bufs=2, space="PSUM") as ps_f1:
                for m in range(64):
                    w1_sb = p_d.tile([128, 16, 128], BF16, name="w1_sb", tag="w1_sb", bufs=3)
                    nc.sync.dma_start(out=w1_sb, in_=w1_in[m].rearrange("a p c -> p a c"))
                    pf = ps_f1.tile([128, TOK], F32, name="pf", tag="pf")
                    for dc in range(16):
                        nc.tensor.matmul(pf, w1_sb[:, dc, :],
                                         h2T_all[:, dc * TOK:(dc + 1) * TOK],
                                         start=(dc == 0), stop=(dc == 15))
                    nc.scalar.activation(out=ff1T[:, m * TOK:(m + 1) * TOK], in_=pf,
                                         func=mybir.ActivationFunctionType.Relu,
                                         bias=b1_t[:, m:m + 1], scale=1.0)
            with tc.tile_pool(name="ps_f2", bufs=1, space="PSUM") as ps_f2:
                for dh in range(2):
                    pd = [ps_f2.tile([128, 512], F32, name=f"pd{i}", tag=f"pd{i}")
                          for i in range(8)]
                    for m in range(64):
                        w2_sb = p_d.tile([128, 1024], BF16, name="w2_sb", tag="w2_sb", bufs=3)
                        nc.sync.dma_start(
                            out=w2_sb,
                            in_=w2_in[m * 128:(m + 1) * 128, dh * 1024:(dh + 1) * 1024])
                        for tb in range(4):
                            for ds in range(2):
                                nc.tensor.matmul(
                                    pd[tb * 2 + ds],
                                    ff1T[:, m * TOK + tb * 128: m * TOK + (tb + 1) * 128],
                                    w2_sb[:, ds * 512:(ds + 1) * 512],
                                    start=(m == 0), stop=(m == 63))
                    for tb in range(4):
                        for ds in range(2):
                            dout = dh * 2 + ds
                            o_sb = p_d.tile([128, 512], F32, name="o_sb", tag="o_sb", bufs=2)
                            nc.vector.tensor_add(
                                out=o_sb, in0=pd[tb * 2 + ds],
                                in1=x2_all[:, tb * D + dout * 512: tb * D + (dout + 1) * 512])
                            nc.sync.dma_start(
                                out=out_ext[tb * 128:(tb + 1) * 128,
                                            dout * 512:(dout + 1) * 512],
                                in_=o_sb)

    # legalize: this walrus build accepts at most ONE sync-wait per instruction
    _legalize_waits(nc, max_waits=1)
    return nc


def _legalize_waits(nc, max_waits=1):
    f = nc.m.functions[0]
    n_split = 0
    for blk in f.blocks:
        insts = blk.instructions
        out = []
        changed = False
        for inst in insts:
            si = inst.sync_info
            if si is not None and si.on_wait and len(si.on_wait) > max_waits:
                waits = list(si.on_wait)
                for w in waits[:-max_waits]:
                    nop = mybir.InstNoOp(
                        name=nc.get_next_instruction_name(),
                        engine=inst.engine,
                        sync_info=mybir.SyncInfo(on_wait=[w], on_update=[]),
                        bass_nofuse=True,
                    )
                    nc.register_instruction(nop)
                    out.append(nop)
                    n_split += 1
                inst.sync_info = mybir.SyncInfo(
                    on_wait=waits[-max_waits:], on_update=list(si.on_update or []))
                changed = True
            out.append(inst)
        if changed:
            blk.instructions = out
    return n_split


def _get_nc():
    if "nc" not in _CACHE:
        _CACHE["nc"] = _build()
    return _CACHE["nc"]


def kernel(**inputs):
    x = np.ascontiguousarray(np.asarray(inputs["x"], np.float32).reshape(B * S, D))
    Wq = np.asarray(inputs["Wq"], np.float32)
    Wk = np.asarray(inputs["Wk"], np.float32)
    Wv = np.asarray(inputs["Wv"], np.float32)
    Wo = np.asarray(inputs["Wo"], np.float32)
    W1 = np.asarray(inputs["W1"], np.float32)
    W2 = np.asarray(inputs["W2"], np.float32)
    b1 = np.asarray(inputs["b1"], np.float32)
    b2 = np.asarray(inputs["b2"], np.float32)
    g1 = np.asarray(inputs["ln1_g"], np.float32)
    c1 = np.asarray(inputs["ln1_b"], np.float32)
    g2 = np.asarray(inputs["ln2_g"], np.float32)
    c2 = np.asarray(inputs["ln2_b"], np.float32)

    wq_p = np.ascontiguousarray(Wq.reshape(16, 128, 16, 128).transpose(2, 0, 1, 3)).astype(BF)
    wk_p = np.ascontiguousarray(Wk.reshape(16, 128, 16, 128).transpose(2, 0, 1, 3)).astype(BF)
    w1_p = np.ascontiguousarray(W1.reshape(16, 128, 64, 128).transpose(2, 0, 1, 3)).astype(BF)
    wv_b = Wv.astype(BF)
    wo_b = Wo.astype(BF)
    w2_b = W2.astype(BF)
    b1t = np.ascontiguousarray(b1.reshape(64, 128).T)
    tri = np.triu(np.full((128, 128), np.float32(-1e9), np.float32), k=1)

    shared = dict(wq=wq_p, wk=wk_p, wv=wv_b, wo=wo_b, w1=w1_p, w2=w2_b,
                  b1t=b1t, b2=b2, ln1g=g1, ln1b=c1, ln2g=g2, ln2b=c2, tri=tri)
    xb = x.reshape(B, S, D)
    in_maps = [dict(shared, x=np.ascontiguousarray(
                   np.concatenate([xb[0, 256 * c:256 * (c + 1)],
                                   xb[1, 256 * c:256 * (c + 1)]], axis=0)))
               for c in range(NC)]

    nc = _get_nc()
    trace = bool(int(os.environ.get("BASS_KERNEL_TRACE", "0")))
    res = run_bass_kernel_spmd(nc, in_maps, list(range(NC)), trace=trace)
    if trace:
        _CACHE["last_exec_time_ns"] = res.exec_time_ns
        _CACHE["last_results"] = res
    out = np.empty((B, S, D), np.float32)
    for c in range(NC):
        oc = res.results[c]["out"]
        out[0, 256 * c:256 * (c + 1)] = oc[0:256]
        out[1, 256 * c:256 * (c + 1)] = oc[256:512]
    return out


# revision 9
# speedup vs baseline: 1.1131x; 1.0175x over previous
"""8-core Trainium2 Bass kernel for nn_DecoderLayer_50783693308094.

Decoder layer: pre-LN attention (with the source model's no-transpose reshape:
"head" n = token-block n of q/k/v reinterpreted (2048, 128); its output is
column block n of the attention output for all tokens) + pre-LN FFN.

Sharding (8 cores): token-parallel everywhere. Core c owns rows
[256c, 256(c+1)) of BOTH batches (= head-blocks 2c, 2c+1 of each batch). LN1 +
QKV + attention are fully local; one 8-core AllToAll redistributes On^T
column-slices; out-projection + residual + LN2 + FFN + residual are
token-local. Matmuls in bf16 (f32 accumulate), everything else f32.
"""
import os
import sys
import numpy as np

if "/opt/trn_rl_repo" not in sys.path:
    sys.path.insert(0, "/opt/trn_rl_repo")

import ml_dtypes
import concourse.bass as bass
import concourse.mybir as mybir
import concourse.tile as tile
from concourse.bass_utils import run_bass_kernel_spmd
from concourse.masks import make_identity
from contextlib import ExitStack

BF = ml_dtypes.bfloat16
F32 = mybir.dt.float32
BF16 = mybir.dt.bfloat16
B, S, D, M, H = 2, 2048, 2048, 8192, 16
d = 128
NC = 8
TOK = 512            # tokens per core
EPS = 1e-5
ISQ = float(1.0 / np.sqrt(np.float32(d)))

_CACHE = {}


def _ln_block(nc, pool, dst_bf16, src_f32, g_bc, b_bc, eps_t):
    """LayerNorm of one [128, D] f32 block -> bf16 into dst_bf16 (same shape)."""
    stats = pool.tile([128, 4, 6], F32, name="ln_stats", tag="ln_stats", bufs=2)
    src_g = src_f32.rearrange("p (s q) -> p s q", s=4)
    for sg in range(4):
        nc.vector.bn_stats(out=stats[:, sg, :], in_=src_g[:, sg, :])
    mv = pool.tile([128, 2], F32, name="ln_mv", tag="ln_mv", bufs=2)
    nc.vector.bn_aggr(out=mv, in_=stats)
    rstd = pool.tile([128, 1], F32, name="ln_rstd", tag="ln_rstd", bufs=2)
    nc.scalar.activation(out=rstd, in_=mv[:, 1:2],
                         func=mybir.ActivationFunctionType.Sqrt,
                         bias=eps_t, scale=1.0)
    nc.vector.reciprocal(out=rstd, in_=rstd)
    h0 = pool.tile([128, D], F32, name="ln_h0", tag="ln_h0", bufs=2)
    nc.vector.tensor_scalar(out=h0, in0=src_f32, scalar1=mv[:, 0:1], scalar2=rstd,
                            op0=mybir.AluOpType.subtract, op1=mybir.AluOpType.mult)
    nc.vector.tensor_mul(out=h0, in0=h0, in1=g_bc)
    nc.vector.tensor_add(out=dst_bf16, in0=h0, in1=b_bc)


def _build():
    nc = bass.Bass(num_devices=NC)

    x_in = nc.declare_dram_parameter("x", [TOK, D], F32, isOutput=False)
    wq_in = nc.declare_dram_parameter("wq", [16, 16, 128, 128], BF16, isOutput=False)
    wk_in = nc.declare_dram_parameter("wk", [16, 16, 128, 128], BF16, isOutput=False)
    wv_in = nc.declare_dram_parameter("wv", [D, D], BF16, isOutput=False)
    wo_in = nc.declare_dram_parameter("wo", [D, D], BF16, isOutput=False)
    w1_in = nc.declare_dram_parameter("w1", [64, 16, 128, 128], BF16, isOutput=False)
    w2_in = nc.declare_dram_parameter("w2", [M, D], BF16, isOutput=False)
    b1_in = nc.declare_dram_parameter("b1t", [128, 64], F32, isOutput=False)
    b2_in = nc.declare_dram_parameter("b2", [D], F32, isOutput=False)
    g1_in = nc.declare_dram_parameter("ln1g", [D], F32, isOutput=False)
    c1_in = nc.declare_dram_parameter("ln1b", [D], F32, isOutput=False)
    g2_in = nc.declare_dram_parameter("ln2g", [D], F32, isOutput=False)
    c2_in = nc.declare_dram_parameter("ln2b", [D], F32, isOutput=False)
    tri_in = nc.declare_dram_parameter("tri", [128, 128], F32, isOutput=False)
    out_ext = nc.declare_dram_parameter("out", [TOK, D], F32, isOutput=True)

    def bcast(ap_1d):
        return bass.AP(tensor=ap_1d.tensor, offset=ap_1d.offset,
                       ap=[[0, 128], list(ap_1d.ap[0])])

    with tile.TileContext(nc) as tc, ExitStack() as ctx:
        misc = ctx.enter_context(tc.tile_pool(name="misc", bufs=1))
        dram = ctx.enter_context(tc.tile_pool(name="dram", bufs=1, space="DRAM"))

        ident = misc.tile([128, 128], BF16)
        make_identity(nc, ident)
        tri_t = misc.tile([128, 128], F32)
        nc.sync.dma_start(out=tri_t, in_=tri_in[:, :])
        b1_t = misc.tile([128, 64], F32)
        nc.sync.dma_start(out=b1_t, in_=b1_in[:, :])
        eps_t = misc.tile([128, 1], F32)
        nc.vector.memset(eps_t, EPS)

        v_scr = dram.tile([TOK, D], BF16, name="v_scr")
        a2a_src0 = dram.tile([NC, 2, 128, 256], BF16, name="a2a_src0")
        a2a_dst0 = dram.tile([NC, 2, 128, 256], BF16, name="a2a_dst0")
        a2a_src1 = dram.tile([NC, 2, 128, 256], BF16, name="a2a_src1")
        a2a_dst1 = dram.tile([NC, 2, 128, 256], BF16, name="a2a_dst1")

        p_x2 = ctx.enter_context(tc.tile_pool(name="p_x2", bufs=1))
        x2_all = p_x2.tile([128, 4 * D], F32)

        p_cd = ctx.enter_context(tc.tile_pool(name="p_cd", bufs=1))
        h2T_all = p_cd.tile([128, 16 * TOK], BF16)

        if True:
            with tc.tile_pool(name="p_qk", bufs=1) as p_qk:
                qT_all = p_qk.tile([128, 16 * TOK], BF16)
                kT_all = p_qk.tile([128, 16 * TOK], BF16)

                # ---------------- phase A: LN1 + QKV ----------------
                with tc.tile_pool(name="p_a", bufs=1) as p_a, \
                     tc.tile_pool(name="ps_t", bufs=2, space="PSUM") as ps_t, \
                     tc.tile_pool(name="ps_qk", bufs=2, space="PSUM") as ps_qk, \
                     tc.tile_pool(name="ps_v", bufs=1, space="PSUM") as ps_v:
                    x_t = []
                    for tb in range(4):
                        xt = p_a.tile([128, D], F32, name=f"x_t{tb}")
                        nc.sync.dma_start(out=xt, in_=x_in[tb * 128:(tb + 1) * 128, :])
                        x_t.append(xt)
                    g1_t = p_a.tile([128, D], F32)
                    nc.sync.dma_start(out=g1_t, in_=bcast(g1_in[:]))
                    c1_t = p_a.tile([128, D], F32)
                    nc.sync.dma_start(out=c1_t, in_=bcast(c1_in[:]))

                    h_t = []
                    for tb in range(4):
                        ht = p_a.tile([128, D], BF16, name=f"h_t{tb}")
                        _ln_block(nc, p_a, ht, x_t[tb], g1_t, c1_t, eps_t)
                        h_t.append(ht)

                    # hT_all[:, dc*512 + tb*128 + p_t] = h[t, dc*128+p]
                    hT_all = p_a.tile([128, 16 * TOK], BF16)
                    for tb in range(4):
                        for dg in range(4):
                            pt = ps_t.tile([128, 512], BF16, name="pt", tag="pt")
                            for z in range(4):
                                dc = dg * 4 + z
                                nc.tensor.transpose(
                                    pt[:, z * 128:(z + 1) * 128],
                                    h_t[tb][:, dc * 128:(dc + 1) * 128],
                                    ident)
                            for z in range(4):
                                dc = dg * 4 + z
                                nc.scalar.copy(
                                    out=hT_all[:, dc * TOK + tb * 128: dc * TOK + (tb + 1) * 128],
                                    in_=pt[:, z * 128:(z + 1) * 128])

                    for w_dram, dstT in ((wq_in, qT_all), (wk_in, kT_all)):
                        for jc in range(16):
                            w_sb = p_a.tile([128, 16, 128], BF16, name="w_sb", tag="w_sb", bufs=3)
                            nc.sync.dma_start(out=w_sb,
                                              in_=w_dram[jc].rearrange("a p c -> p a c"))
                            pq = ps_qk.tile([128, TOK], F32, name="pq", tag="pq")
                            for dc in range(16):
                                nc.tensor.matmul(pq, w_sb[:, dc, :],
                                                 hT_all[:, dc * TOK:(dc + 1) * TOK],
                                                 start=(dc == 0), stop=(dc == 15))
                            nc.scalar.copy(out=dstT[:, jc * TOK:(jc + 1) * TOK], in_=pq)

                    for dv in range(4):
                        pv = [ps_v.tile([128, 512], F32, name=f"pv{tb}", tag=f"pv{tb}")
                              for tb in range(4)]
                        for dc in range(16):
                            wv_sb = p_a.tile([128, 512], BF16, name="wv_sb", tag="wv_sb", bufs=3)
                            nc.sync.dma_start(
                                out=wv_sb,
                                in_=wv_in[dc * 128:(dc + 1) * 128, dv * 512:(dv + 1) * 512])
                            for tb in range(4):
                                nc.tensor.matmul(pv[tb],
                                                 hT_all[:, dc * TOK + tb * 128: dc * TOK + (tb + 1) * 128],
                                                 wv_sb, start=(dc == 0), stop=(dc == 15))
                        for tb in range(4):
                            v_sb = p_a.tile([128, 512], BF16, name="v_sb", tag="v_sb", bufs=2)
                            nc.scalar.copy(out=v_sb, in_=pv[tb])
                            nc.sync.dma_start(
                                out=v_scr[tb * 128:(tb + 1) * 128, dv * 512:(dv + 1) * 512],
                                in_=v_sb)

                # ---------------- phase B: attention ----------------
                qT_r = qT_all.rearrange("p (jc t) -> p t jc", jc=16)
                kT_r = kT_all.rearrange("p (jc t) -> p t jc", jc=16)
                v_flat = v_scr.rearrange("a b -> (a b)")
                with tc.tile_pool(name="ps_proj", bufs=1, space="PSUM") as ps_proj, \
                     tc.tile_pool(name="p_b", bufs=1) as p_b, \
                     tc.tile_pool(name="ps_s", bufs=2, space="PSUM") as ps_s, \
                     tc.tile_pool(name="ps_pt", bufs=2, space="PSUM") as ps_pt, \
                     tc.tile_pool(name="ps_o", bufs=2, space="PSUM") as ps_o:
                    for n in range(4):
                        # materialize interleaved QnT/KnT [dd, i = r*16+jc]
                        qnT = p_b.tile([128, 2048], BF16, name="qnT", tag="qnT", bufs=2)
                        nc.vector.tensor_copy(
                            out=qnT.rearrange("p (r jc) -> p r jc", jc=16),
                            in_=qT_r[:, n * 128:(n + 1) * 128, :])
                        knT = p_b.tile([128, 2048], BF16, name="knT", tag="knT", bufs=2)
                        nc.vector.tensor_copy(
                            out=knT.rearrange("p (r jc) -> p r jc", jc=16),
                            in_=kT_r[:, n * 128:(n + 1) * 128, :])
                        vn_sb = p_b.tile([128, 16 * 128], BF16, name="vn_sb", tag="vn", bufs=2)
                        for kb in range(16):
                            blk = v_flat[n * 128 * D + kb * 16384:
                                         n * 128 * D + (kb + 1) * 16384]
                            nc.sync.dma_start(
                                out=vn_sb[:, kb * 128:(kb + 1) * 128],
                                in_=blk.rearrange("(p c) -> p c", p=128))
                        onT = p_b.tile([128, 2048], BF16, name="onT", tag="onT", bufs=2)
                        for qi in range(16):
                            L = (qi + 1) * 128
                            nch = qi // 4 + 1
                            lhs_q = qnT[:, qi * 128:(qi + 1) * 128]
                            p_all = p_b.tile([128, 2048], BF16, name="p_all", tag="p_all", bufs=2)
                            rsp = p_b.tile([128, 4], F32, name="rsp", tag="rsp", bufs=2)
                            for ci, kc in enumerate(range(0, L, 512)):
                                ncols = min(512, L - kc)
                                ps = ps_s.tile([128, 512], F32, name="ps", tag="ps")
                                nc.tensor.matmul(ps[:, 0:ncols], lhs_q,
                                                 knT[:, kc:kc + ncols],
                                                 start=True, stop=True)
                                if ci == nch - 1:
                                    off = qi * 128 - kc
                                    nc.vector.tensor_add(out=ps[:, off:off + 128],
                                                         in0=ps[:, off:off + 128],
                                                         in1=tri_t)
                                nc.scalar.activation(
                                    out=p_all[:, kc:kc + ncols], in_=ps[:, 0:ncols],
                                    func=mybir.ActivationFunctionType.Exp,
                                    bias=0.0, scale=ISQ,
                                    accum_out=rsp[:, ci:ci + 1])
                            rinv = p_b.tile([128, 1], F32, name="rinv", tag="rinv", bufs=2)
                            nc.vector.reduce_sum(rinv, rsp[:, 0:nch],
                                                 axis=mybir.AxisListType.X)
                            nc.vector.reciprocal(out=rinv, in_=rinv)
                            nc.vector.tensor_scalar_mul(p_all[:, 0:L], p_all[:, 0:L], rinv)
                            pT_all = p_b.tile([128, 2048], BF16, name="pT_all", tag="pT_all", bufs=2)
                            for grp in range(0, qi + 1, 4):
                                gn = min(4, qi + 1 - grp)
                                pt2 = ps_pt.tile([128, 512], BF16, name="pt2", tag="pt2")
                                for z in range(gn):
                                    nc.tensor.transpose(
                                        pt2[:, z * 128:(z + 1) * 128],
                                        p_all[:, (grp + z) * 128:(grp + z + 1) * 128],
                                        ident)
                                nc.vector.tensor_copy(
                                    out=pT_all[:, grp * 128:(grp + gn) * 128],
                                    in_=pt2[:, 0:gn * 128])
                            po = ps_o.tile([128, 128], F32, name="po", tag="po")
                            for kb in range(qi + 1):
                                nc.tensor.matmul(po, vn_sb[:, kb * 128:(kb + 1) * 128],
                                                 pT_all[:, kb * 128:(kb + 1) * 128],
                                                 start=(kb == 0), stop=(kb == qi))
                            nc.scalar.copy(out=onT[:, qi * 128:(qi + 1) * 128], in_=po)
                            if qi % 2 == 1:
                                a2a_s = a2a_src0 if n < 2 else a2a_src1
                                nc.gpsimd.dma_start(
                                    out=a2a_s[qi // 2, n % 2],
                                    in_=onT[:, (qi - 1) * 128:(qi + 1) * 128])
                        if n == 1:
                            nc.gpsimd.collective_compute(
                                "AllToAll", mybir.AluOpType.bypass,
                                replica_groups=[list(range(NC))],
                                ins=[a2a_src0[:, :, :, :]], outs=[a2a_dst0[:, :, :, :]])
                        if n == 3:
                            nc.gpsimd.collective_compute(
                                "AllToAll", mybir.AluOpType.bypass,
                                replica_groups=[list(range(NC))],
                                ins=[a2a_src1[:, :, :, :]], outs=[a2a_dst1[:, :, :, :]])

                    # ------------ phase C proj (inside ps_proj scope) ------------
                    with tc.tile_pool(name="p_c1", bufs=1) as p_c1:
                        wo_done = []
                        for bh in range(2):
                            a2a_d = a2a_dst0 if bh == 0 else a2a_dst1
                            oT_sbh = p_c1.tile([128, 16 * 256], BF16, name=f"oT_sb{bh}")
                            for hc in range(16):
                                nc.sync.dma_start(
                                    out=oT_sbh[:, hc * 256:(hc + 1) * 256],
                                    in_=a2a_d[hc // 2, hc % 2])
                            for dout in range(4):
                                wo_sb = p_c1.tile([128, 16 * 512], BF16,
                                                  name="wo_sb", tag="wo_sb", bufs=2)
                                for hc in range(16):
                                    nc.sync.dma_start(
                                        out=wo_sb[:, hc * 512:(hc + 1) * 512],
                                        in_=wo_in[hc * 128:(hc + 1) * 128,
                                                  dout * 512:(dout + 1) * 512])
                                for tbh in range(2):
                                    tb = bh * 2 + tbh
                                    pc = ps_proj.tile([128, 512], F32,
                                                      name="pc", tag="pc", bufs=2)
                                    for hc in range(16):
                                        nc.tensor.matmul(
                                            pc,
                                            oT_sbh[:, hc * 256 + tbh * 128: hc * 256 + tbh * 128 + 128],
                                            wo_sb[:, hc * 512:(hc + 1) * 512],
                                            start=(hc == 0), stop=(hc == 15))
                                    xr = p_c1.tile([128, 512], F32, name="xr", tag="xr", bufs=2)
                                    nc.sync.dma_start(
                                        out=xr,
                                        in_=x_in[tb * 128:(tb + 1) * 128,
                                                 dout * 512:(dout + 1) * 512])
                                    sl_ = slice(tb * D + dout * 512,
                                                tb * D + (dout + 1) * 512)
                                    nc.vector.tensor_add(out=x2_all[:, sl_],
                                                         in0=pc, in1=xr)

            # ---------------- phase C: LN2 + h2T ----------------
            with tc.tile_pool(name="p_c", bufs=1) as p_c:
                g2_t = p_c.tile([128, D], F32)
                nc.sync.dma_start(out=g2_t, in_=bcast(g2_in[:]))
                c2_t = p_c.tile([128, D], F32)
                nc.sync.dma_start(out=c2_t, in_=bcast(c2_in[:]))
                b2_t = p_c.tile([128, D], F32)
                nc.sync.dma_start(out=b2_t, in_=bcast(b2_in[:]))

                h2_all = p_c.tile([128, 4 * D], BF16)
                for tb in range(4):
                    _ln_block(nc, p_c, h2_all[:, tb * D:(tb + 1) * D],
                              x2_all[:, tb * D:(tb + 1) * D], g2_t, c2_t, eps_t)
                # fold b2 into x2 now (x2b = x2 + b2)
                for tb in range(4):
                    nc.vector.tensor_add(out=x2_all[:, tb * D:(tb + 1) * D],
                                         in0=x2_all[:, tb * D:(tb + 1) * D], in1=b2_t)
                with tc.tile_pool(name="ps_t2", bufs=2, space="PSUM") as ps_t2:
                    for tb in range(4):
                        for dg in range(4):
                            pt3 = ps_t2.tile([128, 512], BF16, name="pt3", tag="pt3")
                            for z in range(4):
                                dc = dg * 4 + z
                                nc.tensor.transpose(
                                    pt3[:, z * 128:(z + 1) * 128],
                                    h2_all[:, tb * D + dc * 128: tb * D + (dc + 1) * 128],
                                    ident)
                            for z in range(4):
                                dc = dg * 4 + z
                                nc.scalar.copy(
                                    out=h2T_all[:, dc * TOK + tb * 128: dc * TOK + (tb + 1) * 128],
                                    in_=pt3[:, z * 128:(z + 1) * 128])

        # ---------------- phase D: FFN ----------------
        with tc.tile_pool(name="p_d", bufs=1) as p_d:
            ff1T = p_d.tile([128, 64 * TOK], BF16)
            with tc.tile_pool(name="ps_f1", bufs=2, space="PSUM") as ps_f1:
                for m in range(64):
                    w1_sb = p_d.tile([128, 16, 128], BF16, name="w1_sb", tag="w1_sb", bufs=3)
                    nc.sync.dma_start(out=w1_sb, in_=w1_in[m].rearrange("a p c -> p a c"))
                    pf = ps_f1.tile([128, TOK], F32, name="pf", tag="pf")
                    for dc in range(16):
                        nc.tensor.matmul(pf, w1_sb[:, dc, :],
                                         h2T_all[:, dc * TOK:(dc + 1) * TOK],
                                         start=(dc == 0), stop=(dc == 15))
                    nc.scalar.activation(out=ff1T[:, m * TOK:(m + 1) * TOK], in_=pf,
                                         func=mybir.ActivationFunctionType.Relu,
                                         bias=b1_t[:, m:m + 1], scale=1.0)
            with tc.tile_pool(name="ps_f2", bufs=1, space="PSUM") as ps_f2:
                for dh in range(2):
                    pd = [ps_f2.tile([128, 512], F32, name=f"pd{i}", tag=f"pd{i}")
                          for i in range(8)]
                    for m in range(64):
                        w2_sb = p_d.tile([128, 1024], BF16, name="w2_sb", tag="w2_sb", bufs=3)
                        nc.sync.dma_start(
                            out=w2_sb,
                            in_=w2_in[m * 128:(m + 1) * 128, dh * 1024:(dh + 1) * 1024])
                        for tb in range(4):
                            for ds in range(2):
                                nc.tensor.matmul(
                                    pd[tb * 2 + ds],
                                    ff1T[:, m * TOK + tb * 128: m * TOK + (tb + 1) * 128],
                                    w2_sb[:, ds * 512:(ds + 1) * 512],
                                    start=(m == 0), stop=(m == 63))
                    for tb in range(4):
                        for ds in range(2):
                            dout = dh * 2 + ds
                            o_sb = p_d.tile([128, 512], F32, name="o_sb", tag="o_sb", bufs=2)
                            nc.vector.tensor_add(
                                out=o_sb, in0=pd[tb * 2 + ds],
                                in1=x2_all[:, tb * D + dout * 512: tb * D + (dout + 1) * 512])
                            nc.sync.dma_start(
                                out=out_ext[tb * 128:(tb + 1) * 128,
                                            dout * 512:(dout + 1) * 512],
                                in_=o_sb)

    # legalize: this walrus build accepts at most ONE sync-wait per instruction
    _legalize_waits(nc, max_waits=1)
    return nc


def _legalize_waits(nc, max_waits=1):
    f = nc.m.functions[0]
    n_split = 0
    for blk in f.blocks:
        insts = blk.instructions
        out = []
        changed = False
        for inst in insts:
            si = inst.sync_info
            if si is not None and si.on_wait and len(si.on_wait) > max_waits:
                waits = list(si.on_wait)
                for w in waits[:-max_waits]:
                    nop = mybir.InstNoOp(
                        name=nc.get_next_instruction_name(),
                        engine=inst.engine,
                        sync_info=mybir.SyncInfo(on_wait=[w], on_update=[]),
                        bass_nofuse=True,
                    )
                    nc.register_instruction(nop)
                    out.append(nop)
                    n_split += 1
                inst.sync_info = mybir.SyncInfo(
                    on_wait=waits[-max_waits:], on_update=list(si.on_update or []))
                changed = True
            out.append(inst)
        if changed:
            blk.instructions = out
    return n_split


def _get_nc():
    if "nc" not in _CACHE:
        _CACHE["nc"] = _build()
    return _CACHE["nc"]


def kernel(**inputs):
    x = np.ascontiguousarray(np.asarray(inputs["x"], np.float32).reshape(B * S, D))
    Wq = np.asarray(inputs["Wq"], np.float32)
    Wk = np.asarray(inputs["Wk"], np.float32)
    Wv = np.asarray(inputs["Wv"], np.float32)
    Wo = np.asarray(inputs["Wo"], np.float32)
    W1 = np.asarray(inputs["W1"], np.float32)
    W2 = np.asarray(inputs["W2"], np.float32)
    b1 = np.asarray(inputs["b1"], np.float32)
    b2 = np.asarray(inputs["b2"], np.float32)
    g1 = np.asarray(inputs["ln1_g"], np.float32)
    c1 = np.asarray(inputs["ln1_b"], np.float32)
    g2 = np.asarray(inputs["ln2_g"], np.float32)
    c2 = np.asarray(inputs["ln2_b"], np.float32)

    wq_p = np.ascontiguousarray(Wq.reshape(16, 128, 16, 128).transpose(2, 0, 1, 3)).astype(BF)
    wk_p = np.ascontiguousarray(Wk.reshape(16, 128, 16, 128).transpose(2, 0, 1, 3)).astype(BF)
    w1_p = np.ascontiguousarray(W1.reshape(16, 128, 64, 128).transpose(2, 0, 1, 3)).astype(BF)
    wv_b = Wv.astype(BF)
    wo_b = Wo.astype(BF)
    w2_b = W2.astype(BF)
    b1t = np.ascontiguousarray(b1.reshape(64, 128).T)
    tri = np.triu(np.full((128, 128), np.float32(-1e9), np.float32), k=1)

    shared = dict(wq=wq_p, wk=wk_p, wv=wv_b, wo=wo_b, w1=w1_p, w2=w2_b,
                  b1t=b1t, b2=b2, ln1g=g1, ln1b=c1, ln2g=g2, ln2b=c2, tri=tri)
    xb = x.reshape(B, S, D)
    in_maps = [dict(shared, x=np.ascontiguousarray(
                   np.concatenate([xb[0, 256 * c:256 * (c + 1)],
                                   xb[1, 256 * c:256 * (c + 1)]], axis=0)))
               for c in range(NC)]

    nc = _get_nc()
    trace = bool(int(os.environ.get("BASS_KERNEL_TRACE", "0")))
    res = run_bass_kernel_spmd(nc, in_maps, list(range(NC)), trace=trace)
    if trace:
        _CACHE["last_exec_time_ns"] = res.exec_time_ns
        _CACHE["last_results"] = res
    out = np.empty((B, S, D), np.float32)
    for c in range(NC):
        oc = res.results[c]["out"]
        out[0, 256 * c:256 * (c + 1)] = oc[0:256]
        out[1, 256 * c:256 * (c + 1)] = oc[256:512]
    return out
